# revision 37
# baseline (speedup 1.0000x reference)
"""Trainium2 Bass kernel for the 2-layer LSTM encoder/decoder problem.

Strategy (8 NeuronCores, tensor-parallel over the 4L=8192 gate rows):
  - Core k owns rows [256k:256k+256) of each gate -> GL=1024 gate rows.
  - Batch 32 = the two independent scan chains of the reference.
  - Gate matmuls are COLUMN-TILED on the PE array: gate q runs on array
    column-group q via tile_position=(0,32q), so four N=256 streams
    execute concurrently and PSUM [128,256] holds gate q at partitions
    [32q:32q+32). Gate order is (i, f, o, g) so one sigmoid activation
    covers partitions 0:96 in a single instruction.
  - Biases are folded into each matmul accumulation group as a K=1
    matmul (ones[1,32] x bias_row[1,256]) - no separate DVE add.
  - The LSTM cell keeps every two-source DVE operand pair base-aligned
    (c at partitions 32:64, tanh(c) at 64:96) to satisfy the SB+SB
    equal-base-partition hardware rule with zero realign copies.
  - enc/consume phases use ONE AllGather per time-step carrying
    [h1_t ; h2_{t-1}]; the AR phase alternates single-h AllGathers.
  - Gathered h is loaded in two chunks on the two HWDGE queues (sync +
    scalar) to halve the small-descriptor drain.
  - Input-side contributions (Wih0 @ x) are bulk matmuls (M=128 over 4
    timesteps); the decoder bulk reads SBUF-resident dWih0, so every
    weight matrix crosses HBM exactly once.
"""

import tempfile

import numpy as np
import ml_dtypes

import concourse.bass as bass  # noqa: F401
import concourse.bacc as bacc
import concourse.mybir as mybir
import concourse.tile as tile
from concourse import bass_utils

C, H, W = 512, 4, 4
SPLIT, PRED = 4, 4
L = 2048
B = 16
NB = 32
NCORES = 8
SL = L // NCORES          # 256
GL = 4 * SL               # 1024 gate rows per core
NT = L // 128             # 16 k-tiles
HT = NT // 2              # 8 k-tiles per load chunk
NAR = PRED + SPLIT - 1    # 7 autoregressive steps

F32 = mybir.dt.float32
BF16 = mybir.dt.bfloat16
NPBF = ml_dtypes.bfloat16

PERM = np.array([4 * (f % C) + f // C for f in range(L)], dtype=np.int64)
IPERM = np.argsort(PERM)
GORDER = [0, 1, 3, 2]     # device gate slots = torch gates (i, f, o, g)

_CACHE = {}


def _build_nc():
    nc = bacc.Bacc("TRN2", target_bir_lowering=False, debug=False,
                   num_devices=NCORES)

    def din(name, shape, dt=F32):
        return nc.dram_tensor(name, shape, dt, kind="ExternalInput").ap()

    def dout(name, shape):
        return nc.dram_tensor(name, shape, F32, kind="ExternalOutput").ap()

    xET = din("xET", [128, NT * 128], BF16)
    xDT = din("xDT", [128, NT * 128], BF16)
    eWih = din("eWih", [2, L, GL], BF16)
    eWhh = din("eWhh", [2, L, GL], BF16)
    dWih = din("dWih", [2, L, GL], BF16)
    dWhh = din("dWhh", [2, L, GL], BF16)
    eBq = din("eBq", [2, 128, SL])     # quadrant-layout bias (f32, for u)
    dBq = din("dBq", [2, 128, SL])
    eBr = din("eBr", [2, 1, GL], BF16)  # bias rows for K=1 matmul fold
    dBr = din("dBr", [2, 1, GL], BF16)
    ones = din("ones", [1, NB], BF16)
    cWT = din("cWT", [2 * C, C], BF16)
    cB = din("cB", [64, C])

    chunks_out = dout("chunks_out", [8, NB, SL])
    convout = dout("convout", [4, 64, C])

    SIG = mybir.ActivationFunctionType.Sigmoid
    TANH = mybir.ActivationFunctionType.Tanh
    RG = [list(range(NCORES))]

    from contextlib import ExitStack
    with tile.TileContext(nc) as tc, ExitStack() as es:
        if True:
            ep = es.enter_context
            wresp = ep(tc.tile_pool(name="wres", bufs=4))
            wstrp = ep(tc.tile_pool(name="wstr", bufs=2))
            xinp = ep(tc.tile_pool(name="xin", bufs=1))
            bqp = ep(tc.tile_pool(name="bq", bufs=2))
            brp = ep(tc.tile_pool(name="br", bufs=5))
            usbp = ep(tc.tile_pool(name="usb", bufs=1))
            uqp = ep(tc.tile_pool(name="uq", bufs=8))
            h1bigp = ep(tc.tile_pool(name="h1big", bufs=8))
            h2bigp = ep(tc.tile_pool(name="h2big", bufs=8))
            convfp = ep(tc.tile_pool(name="convf", bufs=4))
            gsp = ep(tc.tile_pool(name="gs", bufs=1))
            cstp = ep(tc.tile_pool(name="cst", bufs=2))
            cwp = ep(tc.tile_pool(name="cw", bufs=2))
            cwtp = ep(tc.tile_pool(name="cwt", bufs=1))
            cvlp = ep(tc.tile_pool(name="cvl", bufs=2))
            Pp = ep(tc.tile_pool(name="P", bufs=4, space="PSUM"))
            psbp = ep(tc.tile_pool(name="psb", bufs=2, space="PSUM"))
            pcvp = ep(tc.tile_pool(name="pcv", bufs=1, space="PSUM"))
            dramp = ep(tc.tile_pool(name="dram", bufs=4, space="DRAM"))

            # ---------------- small helpers ----------------
            def load_w(w_dram, l, name, eng):
                wt = wresp.tile([128, NT * GL], BF16, tag="wres", name=name)
                eng.dma_start(
                    wt[:].rearrange("p (kt n) -> p kt n", kt=NT),
                    w_dram[l].rearrange("(kt p) n -> p kt n", p=128),
                )
                return wt

            def bias_mm(P, br_sb):
                """Fold bias into the accumulation group: K=1 matmul of
                ones[1,32] x bias_row[1,256] per gate (start=True)."""
                for q in range(4):
                    nc.tensor.matmul(
                        P[32 * q:32 * q + 32, :],
                        ones_sb[0:1, :],
                        br_sb[0:1, SL * q:SL * q + SL],
                        start=True, stop=False,
                        tile_position=(0, 32 * q),
                        skip_group_check=True)

            def qmm(P, lhs_fn, w_sb, start, stop, order=None):
                """Col-tiled gate matmuls: gate q on array col-group q."""
                if order is None:
                    order = range(NT)
                for i, kt in enumerate(order):
                    lhs = lhs_fn(kt)
                    for q in range(4):
                        nc.tensor.matmul(
                            P[32 * q:32 * q + 32, :],
                            lhs,
                            w_sb[:, kt * GL + SL * q: kt * GL + SL * q + SL],
                            start=(start and i == 0),
                            stop=(stop and i == NT - 1),
                            tile_position=(0, 32 * q),
                            skip_group_check=True)

            def cellq(G, u_ap, c_old, ltag, out_idx=None):
                """LSTM cell from quadrant gates (i,f,o,g order).
                G: [128, SL] AP (PSUM with bias folded, or SBUF u tile).
                u_ap: optional extra [128, SL] SBUF add (enc/cons l0).
                c state tiles are [64, SL] with data at rows 32:64."""
                act = nc.scalar.activation
                if u_ap is not None:
                    gs = gsp.tile([128, SL], F32, tag="gs")
                    nc.vector.tensor_add(gs[:], G, u_ap)
                    G = gs[:]
                sifo = cwp.tile([96, SL], F32, tag="sifo")
                act(sifo[:], G[0:96, :], SIG)
                tg = cwp.tile([32, SL], F32, tag="tg")
                act(tg[:], G[96:128, :], TANH)
                c_new = cstp.tile([64, SL], F32, tag="c" + ltag)
                if c_old is not None:
                    # cm first: it only needs sifo, runs during the tg ACT
                    cm = cwp.tile([64, SL], F32, tag="cm")
                    nc.vector.tensor_mul(cm[32:64, :], sifo[32:64, :],
                                         c_old[32:64, :])
                tmp = cwp.tile([64, SL], F32, tag="tmp")
                nc.vector.tensor_mul(tmp[32:64, :], sifo[0:32, :], tg[:])
                if c_old is not None:
                    nc.vector.tensor_add(c_new[32:64, :], cm[32:64, :],
                                         tmp[32:64, :])
                else:
                    nc.vector.tensor_copy(c_new[32:64, :], tmp[32:64, :])
                tct = cwp.tile([96, SL], F32, tag="tct")
                act(tct[64:96, :], c_new[32:64, :], TANH)
                hb = cwp.tile([96, SL], BF16, tag="hb")
                nc.vector.tensor_mul(hb[64:96, :], sifo[64:96, :],
                                     tct[64:96, :])
                ht = cwp.tile([32, SL], BF16, tag="ht")
                nc.vector.transpose(ht[:], hb[64:96, :])
                if out_idx is not None:
                    h32 = cwp.tile([96, SL], F32, tag="h32")
                    nc.vector.tensor_mul(h32[64:96, :], sifo[64:96, :],
                                         tct[64:96, :])
                    nc.sync.dma_start(chunks_out[out_idx], h32[64:96, :])
                return c_new, ht

            def write_ht(cin, ht, half):
                nc.sync.dma_start(
                    cin[half * SL:(half + 1) * SL].rearrange(
                        "(q j) b -> j q b", j=32),
                    ht[:].rearrange("j (q b) -> j q b", b=NB))

            def ag(cin, nrows, name):
                cout = dramp.tile([nrows * NCORES, NB], BF16,
                                  tag=f"cout{nrows}", name="co_" + name,
                                  addr_space="Shared")
                nc.gpsimd.collective_compute(
                    "AllGather", mybir.AluOpType.bypass,
                    replica_groups=RG, ins=[cin[:]], outs=[cout[:]])
                return cout

            def load_q1(cout, pool, tag):
                """Single-h cout [2048,32] -> 4 chunk tiles of 4 kt,
                pipelined on alternating queues. Returns kt->(tile,idx)
                plus the kt consumption order."""
                fns = {}
                src = cout.rearrange("(kt p) b -> p kt b", p=128)
                for c in range(4):
                    eng = nc.sync if c % 2 == 0 else nc.scalar
                    t_ = pool.tile([128, 4 * NB], BF16, tag=tag)
                    eng.dma_start(
                        t_[:].rearrange("p (kt b) -> p kt b", kt=4),
                        src[:, 4 * c:4 * c + 4, :])
                    for i in range(4):
                        fns[4 * c + i] = (t_, i)
                return fns, list(range(NT))

            def load_q2(cout, half, pool, tag, eng):
                """Combined cout [4096,32]; chunk (s,rh) holds
                kt = 2*(4*rh+r)+s."""
                fns = {}
                order = []
                src = cout.rearrange("(r h s p) b -> h s p r b",
                                     h=2, s=2, p=128)[half]
                for (s, rh) in [(0, 0), (1, 0), (0, 1), (1, 1)]:
                    t_ = pool.tile([128, 4 * NB], BF16, tag=tag)
                    eng.dma_start(
                        t_[:].rearrange("p (r b) -> p r b", r=4),
                        src[s][:, 4 * rh:4 * rh + 4, :])
                    for r_ in range(4):
                        kt = 2 * (4 * rh + r_) + s
                        fns[kt] = (t_, r_)
                        order.append(kt)
                return fns, order

            def fns_lhs(fns):
                def f(kt):
                    t_, i = fns[kt]
                    return t_[:, i * NB:i * NB + NB]
                return f

            def assemble_full(fns, pool, name):
                # 4 contiguous-kt chunk tiles -> one [128, NT*NB] tile
                ft = pool.tile([128, NT * NB], BF16, tag="convf",
                               name=name)
                for c in range(4):
                    t_ = fns[4 * c][0]
                    nc.vector.tensor_copy(
                        ft[:, c * 4 * NB:(c + 1) * 4 * NB], t_[:])
                return ft

            # ---------------- bulk input matmuls ----------------
            def finish_bulk(psA, psB, bq_ap, nm):
                u_sb = usbp.tile([128, GL], F32, tag="usb", name=nm + "u")
                nc.vector.tensor_copy(u_sb[:, 0:512], psA[:])
                nc.vector.tensor_copy(u_sb[:, 512:GL], psB[:])
                uts = []
                for t in range(4):
                    uq = uqp.tile([128, SL], F32, tag="uq",
                                  name=f"{nm}uq{t}")
                    for q in range(4):
                        nc.sync.dma_start(
                            uq[32 * q:32 * q + 32, :],
                            u_sb[32 * t:32 * t + 32,
                                 SL * q:SL * q + SL])
                    nc.vector.tensor_add(uq[:], uq[:], bq_ap)
                    uts.append(uq)
                return uts

            def bulk(x_sb, rhs_fn, bq_ap, nm):
                psA = psbp.tile([128, 512], F32, tag="psb", name=nm + "A")
                psB = psbp.tile([128, 512], F32, tag="psb", name=nm + "B")
                for kt in range(NT):
                    r0, r1 = rhs_fn(kt)
                    lhs = x_sb[:, kt * 128:(kt + 1) * 128]
                    nc.tensor.matmul(psA[:], lhs, r0,
                                     start=(kt == 0), stop=(kt == NT - 1),
                                     skip_group_check=True)
                    nc.tensor.matmul(psB[:], lhs, r1,
                                     start=(kt == 0), stop=(kt == NT - 1),
                                     skip_group_check=True)
                return finish_bulk(psA, psB, bq_ap, nm)

            # ---------------- conv emit (from gathered h2 pairs) --------
            def emit_conv(cwt_sb, cb_sb, conv_tiles):
                b1 = [conv_tiles[0], conv_tiles[1], conv_tiles[2],
                      conv_tiles[3]]
                b2 = [conv_tiles[3], conv_tiles[2], conv_tiles[1],
                      conv_tiles[0]]
                for w in range(4):
                    pcv = pcvp.tile([128, 512], F32, tag="pcv",
                                    name=f"pcv{w}")
                    first = True
                    for br, src in ((0, b1[w]), (1, b2[w])):
                        lhs = src[:].rearrange("p (kt b) -> p kt b", kt=NT)
                        for j in range(4):
                            st = cvlp.tile([128, 64], BF16, tag="cvl",
                                           name=f"cvl{w}_{br}_{j}")
                            nc.vector.tensor_copy(
                                st[:].rearrange("p (h b) -> p h b", h=4),
                                lhs[:, j::4, 16 * br:16 * br + 16])
                            nc.tensor.matmul(
                                pcv[0:64, :], st[:],
                                cwt_sb[:, (4 * br + j) * C:
                                       (4 * br + j + 1) * C],
                                start=first, stop=(br == 1 and j == 3),
                                skip_group_check=True)
                            first = False
                    cvs = usbp.tile([64, C], F32, tag="cvs", name=f"cvs{w}")
                    nc.vector.tensor_add(cvs[:], pcv[0:64, :], cb_sb[:])
                    cvo = usbp.tile([64, C], F32, tag="cvo", name=f"cvo{w}")
                    nc.vector.tensor_scalar_mul(cvo[:], cvs[:], 0.2)
                    nc.vector.tensor_max(cvo[:], cvo[:], cvs[:])
                    nc.sync.dma_start(convout[w], cvo[:])

            # =========================================================
            # Preamble: input/weight DMAs on the two HWDGE queues
            # =========================================================
            xe_sb = xinp.tile([128, NT * 128], BF16, tag="xin", name="xe")
            nc.scalar.dma_start(xe_sb[:], xET[:, :])
            xd_sb = xinp.tile([128, NT * 128], BF16, tag="xin", name="xd")
            nc.scalar.dma_start(xd_sb[:], xDT[:, :])

            def load_bq(src, l, name):
                t_ = bqp.tile([128, SL], F32, tag="bq", name=name)
                nc.scalar.dma_start(t_[:], src[l])
                return t_

            eb0 = load_bq(eBq, 0, "eb0")
            db0 = load_bq(dBq, 0, "db0")

            ones_sb = brp.tile([1, NB], BF16, tag="ones", name="ones_sb")
            nc.scalar.dma_start(ones_sb[:], ones[:, :])

            def load_br(src, l, name):
                t_ = brp.tile([1, GL], BF16, tag="br", name=name)
                nc.scalar.dma_start(t_[:], src[l])
                return t_

            ebr1 = load_br(eBr, 1, "ebr1")
            dbr0 = load_br(dBr, 0, "dbr0")
            dbr1 = load_br(dBr, 1, "dbr1")

            # weights: scalar queue = enc stream + remaining residents;
            # sync queue = whh_e0 in parallel (it idles at startup).
            whh_e0 = load_w(eWhh, 0, "whh_e0", nc.sync)

            # enc bulk: stream eWih0 k-tile slabs on the scalar queue
            wstr_tiles = []
            for kt in range(NT):
                wt = wstrp.tile([128, GL], BF16, tag="wstr", name=f"ws{kt}")
                nc.scalar.dma_start(
                    wt[:], eWih[0, kt * 128:(kt + 1) * 128, :])
                wstr_tiles.append(wt)
            ue = bulk(xe_sb,
                      lambda kt: (wstr_tiles[kt][:, 0:512],
                                  wstr_tiles[kt][:, 512:GL]),
                      eb0[:], "ue")

            wih_e1 = load_w(eWih, 1, "wih_e1", nc.scalar)
            whh_e1 = load_w(eWhh, 1, "whh_e1", nc.scalar)
            wih_d0 = load_w(dWih, 0, "wih_d0", nc.scalar)
            whh_d0 = load_w(dWhh, 0, "whh_d0", nc.scalar)
            wih_d1 = load_w(dWih, 1, "wih_d1", nc.scalar)
            whh_d1 = load_w(dWhh, 1, "whh_d1", nc.scalar)

            cb_sb = bqp.tile([64, C], F32, tag="cb", name="cb_sb")
            nc.scalar.dma_start(cb_sb[:], cB[:])
            cwt_sb = cwtp.tile([128, 8 * C], BF16, tag="cwt")
            nc.scalar.dma_start(
                cwt_sb[:].rearrange("p (j o) -> p j o", j=8),
                cWT.rearrange("(j p) o -> p j o", p=128))

            # =========================================================
            # Phase E + C: wavefront, one combined AG per time-step
            # =========================================================
            h1big = None      # gathered h1_t chunk pair
            h2big = None      # gathered h2_{t-1} chunk pair
            c1 = c2 = None
            cin_next = None
            ud = None

            for phase in ("E", "C"):
                if phase == "E":
                    uts, w0, w1i, w1h, br1 = ue, whh_e0, wih_e1, whh_e1, ebr1
                else:
                    uts, w0, w1i, w1h, br1 = ud, whh_d0, wih_d1, whh_d1, dbr1

                for t in range(4):
                    first = (phase == "E" and t == 0)
                    # ---- layer 0 step t ----
                    if first:
                        c1, ht1 = cellq(uts[0][:], None, None, "1")
                    else:
                        P0 = Pp.tile([128, SL], F32, tag="P",
                                     name=f"P0{phase}{t}")
                        qmm(P0, fns_lhs(h1big), w0, start=True, stop=True,
                            order=h1ord)
                        c1, ht1 = cellq(P0[:], uts[t][:], c1, "1")
                    if first:
                        cin = dramp.tile([SL, NB], BF16, tag="cin1",
                                         name="cinE0")
                    else:
                        cin = cin_next
                    write_ht(cin, ht1, 0)
                    cout = ag(cin, SL if first else 2 * SL, f"{phase}{t}")

                    # dec bulk (reads resident dWih0) rides the E windows
                    if phase == "E" and t == 2:
                        ud = bulk(xd_sb,
                                  lambda kt: (wih_d0[:, kt * GL:
                                                     kt * GL + 512],
                                              wih_d0[:, kt * GL + 512:
                                                     (kt + 1) * GL]),
                                  db0[:], "ud")

                    # ---- AG results ----
                    if first:
                        h1big, h1ord = load_q1(cout, h1bigp, "h1big")
                    else:
                        h1big, h1ord = load_q2(cout, 0, h1bigp, "h1big",
                                               nc.sync)
                        h2big, h2ord = load_q2(cout, 1, h2bigp, "h2big",
                                               nc.scalar)

                    # ---- layer 1 step t ----
                    P1 = Pp.tile([128, SL], F32, tag="P",
                                 name=f"P1{phase}{t}")
                    bias_mm(P1, br1)
                    if first:
                        qmm(P1, fns_lhs(h1big), w1i, start=False,
                            stop=True, order=h1ord)
                        c2, ht2 = cellq(P1[:], None, None, "2")
                    else:
                        qmm(P1, fns_lhs(h2big), w1h, start=False,
                            stop=False, order=h2ord)
                        qmm(P1, fns_lhs(h1big), w1i, start=False,
                            stop=True, order=h1ord)
                        oi = 0 if (phase == "C" and t == 3) else None
                        c2, ht2 = cellq(P1[:], None, c2, "2", out_idx=oi)
                    # write h2_t into the NEXT AG's cin
                    if phase == "C" and t == 3:
                        cin_next = dramp.tile([SL, NB], BF16, tag="cin1",
                                              name="cinAR0")
                        write_ht(cin_next, ht2, 0)
                    else:
                        cin_next = dramp.tile([2 * SL, NB], BF16,
                                              tag="cin2",
                                              name=f"cin{phase}{t + 1}")
                        write_ht(cin_next, ht2, 1)

            # =========================================================
            # Phase AR: alternating single-h AllGathers
            # =========================================================
            conv_tiles = []
            for t in range(NAR):
                # ---- AG a(t): gather h2_{t-1} ----
                P0 = Pp.tile([128, SL], F32, tag="P", name=f"P0a{t}")
                with tc.high_priority(offset=2000):
                    bias_mm(P0, dbr0)
                    qmm(P0, fns_lhs(h1big), whh_d0, start=False,
                        stop=False, order=h1ord)
                cout = ag(cin_next, SL, f"a{t}")
                h2big, h2ord = load_q1(cout, h2bigp, "h2big")
                if t < 4:
                    conv_tiles.append(
                        assemble_full(h2big, convfp, f"cf{t}"))
                qmm(P0, fns_lhs(h2big), wih_d0, start=False, stop=True,
                    order=h2ord)
                c1, ht1 = cellq(P0[:], None, c1, "1")
                cin_next = dramp.tile([SL, NB], BF16, tag="cin1",
                                      name=f"cinb{t}")
                write_ht(cin_next, ht1, 0)

                # ---- AG b(t): gather h1 of this step ----
                P1 = Pp.tile([128, SL], F32, tag="P", name=f"P1a{t}")
                with tc.high_priority(offset=2000):
                    bias_mm(P1, dbr1)
                    qmm(P1, fns_lhs(h2big), whh_d1, start=False,
                        stop=False, order=h2ord)
                cout = ag(cin_next, SL, f"b{t}")
                if t == 3:
                    emit_conv(cwt_sb, cb_sb, conv_tiles)
                h1big, h1ord = load_q1(cout, h1bigp, "h1big")
                qmm(P1, fns_lhs(h1big), wih_d1, start=False, stop=True,
                    order=h1ord)
                c2, ht2 = cellq(P1[:], None, c2, "2", out_idx=t + 1)
                if t < NAR - 1:
                    cin_next = dramp.tile([SL, NB], BF16, tag="cin1",
                                          name=f"cina{t + 1}")
                    write_ht(cin_next, ht2, 0)

    nc.compile()
    return nc


def _prep_inputs(x1, x2, enc_Wih, enc_Whh, enc_bih, enc_bhh,
                 dec_Wih, dec_Whh, dec_bih, dec_bhh, conv_W, conv_b):
    def colvecs(x):
        return [np.ascontiguousarray(x[:, :, :, t].reshape(B, L))
                for t in range(4)]

    x1c, x2c = colvecs(x1), colvecs(x2)

    def ximg(xa):
        # [4, L, NB] -> SBUF image [128, kt*128 + t*32 + b]
        return np.ascontiguousarray(
            xa.reshape(4, NT, 128, NB).transpose(2, 1, 0, 3)
            .reshape(128, NT * 4 * NB)).astype(NPBF)

    xET = ximg(np.stack([
        np.concatenate([x2c[t], x1c[3 - t]], axis=0)[:, PERM].T
        for t in range(4)]))
    xDT = ximg(np.stack([
        np.concatenate([x1c[t], x2c[3 - t]], axis=0)[:, PERM].T
        for t in range(4)]))

    def prep_core(k, Wih, Whh, bih, bhh):
        rows = np.concatenate([g * L + PERM[k * SL:(k + 1) * SL]
                               for g in GORDER])
        wihT = np.stack([np.ascontiguousarray(Wih[l][rows][:, PERM].T)
                         for l in range(2)])
        whhT = np.stack([np.ascontiguousarray(Whh[l][rows][:, PERM].T)
                         for l in range(2)])
        bb = np.stack([(bih[l] + bhh[l])[rows] for l in range(2)])
        # quadrant layout: [l, 4 gates x 32 batch, SL]
        bq = np.broadcast_to(
            bb.reshape(2, 4, 1, SL), (2, 4, 32, SL)).reshape(2, 128, SL)
        br = bb.reshape(2, 1, GL)
        return (wihT.astype(NPBF), whhT.astype(NPBF),
                np.ascontiguousarray(bq).astype(np.float32),
                np.ascontiguousarray(br).astype(NPBF))

    cWT = np.ascontiguousarray(conv_W.T).astype(NPBF)
    cBr = np.broadcast_to(conv_b[None, :], (64, C)).copy().astype(np.float32)
    ones_h = np.ones((1, NB), dtype=NPBF)

    in_maps = []
    for k in range(NCORES):
        eWihT, eWhhT, eBq_, eBr_ = prep_core(
            k, enc_Wih, enc_Whh, enc_bih, enc_bhh)
        dWihT, dWhhT, dBq_, dBr_ = prep_core(
            k, dec_Wih, dec_Whh, dec_bih, dec_bhh)
        in_maps.append({
            "xET": xET, "xDT": xDT,
            "eWih": eWihT, "eWhh": eWhhT, "eBq": eBq_, "eBr": eBr_,
            "dWih": dWihT, "dWhh": dWhhT, "dBq": dBq_, "dBr": dBr_,
            "ones": ones_h, "cWT": cWT, "cB": cBr,
        })
    return in_maps


def _postprocess(results, x1, x2):
    chunks = np.zeros((8, B * 2, L), np.float32)
    for k in range(NCORES):
        chunks[:, :, k * SL:(k + 1) * SL] = results[k]["chunks_out"]
    convout = results[0]["convout"]

    def tochunk(t, half):
        v = chunks[t, half * B:(half + 1) * B, :]
        return v[:, IPERM].reshape(B, C, H)

    de1 = tochunk(0, 0)
    p1 = [tochunk(1 + j, 0) for j in range(NAR)]
    de2 = tochunk(0, 1)
    p2 = [tochunk(1 + j, 1) for j in range(NAR)]

    mid1 = np.stack([de1, p1[0], p1[1], p1[2]], axis=-1)
    tail1 = np.stack([p1[3], p1[4], p1[5], p1[6]], axis=-1)
    head2 = np.stack([p2[6], p2[5], p2[4], p2[3]], axis=-1)
    mid2 = np.stack([p2[2], p2[1], p2[0], de2], axis=-1)

    out = convout.reshape(4, 4, B, C).transpose(2, 3, 1, 0)
    out = np.ascontiguousarray(out, dtype=np.float32)
    return (out, np.asarray(x1), mid1, tail1, head2, mid2, np.asarray(x2))


def _run(in_maps, trace=False):
    if "nc" not in _CACHE:
        _CACHE["nc"] = _build_nc()
        _CACHE["tmpdir"] = tempfile.mkdtemp(prefix="lstmk_")
    nc = _CACHE["nc"]
    res = bass_utils.run_bass_kernel_spmd(
        nc, in_maps, core_ids=list(range(NCORES)), trace=trace,
        tmpdir=_CACHE["tmpdir"] if trace else None)
    return res


def kernel(**inputs):
    inputs = {k: np.asarray(v, dtype=np.float32) for k, v in inputs.items()}
    in_maps = _prep_inputs(**inputs)
    res = _run(in_maps, trace=False)
    return _postprocess(res.results, inputs["x1"], inputs["x2"])


def kernel_traced(**inputs):
    inputs = {k: np.asarray(v, dtype=np.float32) for k, v in inputs.items()}
    in_maps = _prep_inputs(**inputs)
    res = _run(in_maps, trace=True)
    return _postprocess(res.results, inputs["x1"], inputs["x2"]), res


# revision 46
# speedup vs baseline: 1.2233x; 1.2233x over previous
"""Trainium2 Bass kernel for the 2-layer LSTM encoder/decoder problem.

Strategy (8 NeuronCores, tensor-parallel over the 4L=8192 gate rows):
  - Core k owns rows [256k:256k+256) of each gate -> GL=1024 gate rows.
  - Batch 32 = the two independent scan chains of the reference.
  - Gate matmuls are COLUMN-TILED on the PE array: gate q runs on array
    column-group q via tile_position=(0,32q), so four N=256 streams
    execute concurrently and PSUM [128,256] holds gate q at partitions
    [32q:32q+32). Gate order is (i, f, o, g) so one sigmoid activation
    covers partitions 0:96 in a single instruction.
  - Biases are folded into each matmul accumulation group as a K=1
    matmul (ones[1,32] x bias_row[1,256]) - no separate DVE add.
  - The LSTM cell keeps every two-source DVE operand pair base-aligned
    (c at partitions 32:64, tanh(c) at 64:96) to satisfy the SB+SB
    equal-base-partition hardware rule with zero realign copies.
  - enc/consume phases use ONE AllGather per time-step carrying
    [h1_t ; h2_{t-1}]; the AR phase alternates single-h AllGathers.
  - Gathered h is loaded in two chunks on the two HWDGE queues (sync +
    scalar) to halve the small-descriptor drain.
  - Input-side contributions (Wih0 @ x) are bulk matmuls (M=128 over 4
    timesteps); the decoder bulk reads SBUF-resident dWih0, so every
    weight matrix crosses HBM exactly once.
"""

import tempfile

import numpy as np
import ml_dtypes

import concourse.bass as bass  # noqa: F401
import concourse.bacc as bacc
import concourse.mybir as mybir
import concourse.tile as tile
from concourse import bass_utils

C, H, W = 512, 4, 4
SPLIT, PRED = 4, 4
L = 2048
B = 16
NB = 32
NCORES = 8
SL = L // NCORES          # 256
GL = 4 * SL               # 1024 gate rows per core
NT = L // 128             # 16 k-tiles
HT = NT // 2              # 8 k-tiles per load chunk
NAR = PRED + SPLIT - 1    # 7 autoregressive steps

F32 = mybir.dt.float32
BF16 = mybir.dt.bfloat16
NPBF = ml_dtypes.bfloat16

PERM = np.array([4 * (f % C) + f // C for f in range(L)], dtype=np.int64)
IPERM = np.argsort(PERM)
GORDER = [0, 1, 3, 2]     # device gate slots = torch gates (i, f, o, g)

_CACHE = {}


def _build_nc():
    nc = bacc.Bacc("TRN2", target_bir_lowering=False, debug=False,
                   num_devices=NCORES)

    def din(name, shape, dt=F32):
        return nc.dram_tensor(name, shape, dt, kind="ExternalInput").ap()

    def dout(name, shape):
        return nc.dram_tensor(name, shape, F32, kind="ExternalOutput").ap()

    xET = din("xET", [128, NT * 128], BF16)
    xDT = din("xDT", [128, NT * 128], BF16)
    eWih = din("eWih", [2, L, GL], BF16)
    eWhh = din("eWhh", [2, L, GL], BF16)
    dWih = din("dWih", [2, L, GL], BF16)
    dWhh = din("dWhh", [2, L, GL], BF16)
    eBq = din("eBq", [2, 128, SL])     # quadrant-layout bias (f32, for u)
    dBq = din("dBq", [2, 128, SL])
    eBr = din("eBr", [2, 1, GL], BF16)  # bias rows for K=1 matmul fold
    dBr = din("dBr", [2, 1, GL], BF16)
    ones = din("ones", [1, NB], BF16)
    cWT = din("cWT", [2 * C, C], BF16)
    cB = din("cB", [64, C])

    chunks_out = dout("chunks_out", [8, NB, SL])
    convout = dout("convout", [4, 64, C])

    SIG = mybir.ActivationFunctionType.Sigmoid
    TANH = mybir.ActivationFunctionType.Tanh
    RG = [list(range(NCORES))]

    from contextlib import ExitStack
    with tile.TileContext(nc) as tc, ExitStack() as es:
        if True:
            ep = es.enter_context
            wresp = ep(tc.tile_pool(name="wres", bufs=4))
            wstrp = ep(tc.tile_pool(name="wstr", bufs=2))
            xinp = ep(tc.tile_pool(name="xin", bufs=2))
            bqp = ep(tc.tile_pool(name="bq", bufs=2))
            brp = ep(tc.tile_pool(name="br", bufs=5))
            usbp = ep(tc.tile_pool(name="usb", bufs=1))
            uqp = ep(tc.tile_pool(name="uq", bufs=8))
            h1bigp = ep(tc.tile_pool(name="h1big", bufs=8))
            h2bigp = ep(tc.tile_pool(name="h2big", bufs=8))
            convfp = ep(tc.tile_pool(name="convf", bufs=4))
            gsp = ep(tc.tile_pool(name="gs", bufs=1))
            cstp = ep(tc.tile_pool(name="cst", bufs=2))
            cwp = ep(tc.tile_pool(name="cw", bufs=1))
            hq8p = ep(tc.tile_pool(name="hq8", bufs=4))
            cwtp = ep(tc.tile_pool(name="cwt", bufs=1))
            cvlp = ep(tc.tile_pool(name="cvl", bufs=2))
            Pp = ep(tc.tile_pool(name="P", bufs=2, space="PSUM"))
            psbp = ep(tc.tile_pool(name="psb", bufs=2, space="PSUM"))
            pcvp = ep(tc.tile_pool(name="pcv", bufs=1, space="PSUM"))
            dramp = ep(tc.tile_pool(name="dram", bufs=4, space="DRAM"))

            # ---------------- small helpers ----------------
            def load_w(w_dram, l, name, eng):
                wt = wresp.tile([128, NT * GL], BF16, tag="wres", name=name)
                eng.dma_start(
                    wt[:].rearrange("p (kt n) -> p kt n", kt=NT),
                    w_dram[l].rearrange("(kt p) n -> p kt n", p=128),
                )
                return wt

            def bias_mm(P, br_sb):
                """Fold bias into the accumulation group: K=1 matmul of
                ones[1,32] x bias_row[1,256] per gate (start=True)."""
                for q in range(4):
                    nc.tensor.matmul(
                        P[32 * q:32 * q + 32, :],
                        ones_sb[0:1, :],
                        br_sb[0:1, SL * q:SL * q + SL],
                        start=True, stop=False,
                        tile_position=(0, 32 * q),
                        skip_group_check=True)

            def qmm(P, lhs_fn, w_sb, start, stop, order=None):
                """Col-tiled gate matmuls: gate q on array col-group q."""
                if order is None:
                    order = range(NT)
                for i, kt in enumerate(order):
                    lhs = lhs_fn(kt)
                    for q in range(4):
                        nc.tensor.matmul(
                            P[32 * q:32 * q + 32, :],
                            lhs,
                            w_sb[:, kt * GL + SL * q: kt * GL + SL * q + SL],
                            start=(start and i == 0),
                            stop=(stop and i == NT - 1),
                            tile_position=(0, 32 * q),
                            skip_group_check=True)

            def cellq(G, u_ap, c_old, ltag, out_idx=None):
                """LSTM cell from quadrant gates (i,f,o,g order).
                G: [128, SL] AP (PSUM with bias folded, or SBUF u tile).
                u_ap: optional extra [128, SL] SBUF add (enc/cons l0).
                c state tiles are [64, SL] with data at rows 32:64."""
                act = nc.scalar.activation
                if u_ap is not None:
                    gs = gsp.tile([128, SL], F32, tag="gs")
                    nc.vector.tensor_add(gs[:], G, u_ap)
                    G = gs[:]
                sifo = cwp.tile([96, SL], F32, tag="sifo")
                act(sifo[:], G[0:96, :], SIG)
                tg = cwp.tile([32, SL], F32, tag="tg")
                act(tg[:], G[96:128, :], TANH)
                c_new = cstp.tile([64, SL], F32, tag="c" + ltag)
                if c_old is not None:
                    # cm first: it only needs sifo, runs during the tg ACT
                    cm = cwp.tile([64, SL], F32, tag="cm")
                    nc.vector.tensor_mul(cm[32:64, :], sifo[32:64, :],
                                         c_old[32:64, :])
                tmp = cwp.tile([64, SL], F32, tag="tmp")
                nc.vector.tensor_mul(tmp[32:64, :], sifo[0:32, :], tg[:])
                if c_old is not None:
                    nc.vector.tensor_add(c_new[32:64, :], cm[32:64, :],
                                         tmp[32:64, :])
                else:
                    nc.vector.tensor_copy(c_new[32:64, :], tmp[32:64, :])
                tct = cwp.tile([96, SL], F32, tag="tct")
                act(tct[64:96, :], c_new[32:64, :], TANH)
                hb = cwp.tile([96, SL], BF16, tag="hb")
                nc.vector.tensor_mul(hb[64:96, :], sifo[64:96, :],
                                     tct[64:96, :])
                ht = cwp.tile([32, SL], BF16, tag="ht")
                nc.vector.transpose(ht[:], hb[64:96, :])
                if out_idx is not None:
                    h32 = cwp.tile([96, SL], F32, tag="h32")
                    nc.vector.tensor_mul(h32[64:96, :], sifo[64:96, :],
                                         tct[64:96, :])
                    nc.sync.dma_start(chunks_out[out_idx], h32[64:96, :])
                return c_new, ht

            def write_ht(cin, ht, half):
                nc.sync.dma_start(
                    cin[half * SL:(half + 1) * SL].rearrange(
                        "(q j) b -> j q b", j=32),
                    ht[:].rearrange("j (q b) -> j q b", b=NB))

            def ag(cin, nrows, name):
                cout = dramp.tile([nrows * NCORES, NB], BF16,
                                  tag=f"cout{nrows}", name="co_" + name,
                                  addr_space="Shared")
                nc.gpsimd.collective_compute(
                    "AllGather", mybir.AluOpType.bypass,
                    replica_groups=RG, ins=[cin[:]], outs=[cout[:]])
                return cout

            def load_q1(cout, pool, tag):
                """Single-h cout [2048,32] -> 4 chunk tiles of 4 kt,
                pipelined on alternating queues. Returns kt->(tile,idx)
                plus the kt consumption order."""
                fns = {}
                src = cout.rearrange("(kt p) b -> p kt b", p=128)
                for c in range(4):
                    eng = nc.sync if c % 2 == 0 else nc.scalar
                    t_ = pool.tile([128, 4 * NB], BF16, tag=tag)
                    eng.dma_start(
                        t_[:].rearrange("p (kt b) -> p kt b", kt=4),
                        src[:, 4 * c:4 * c + 4, :])
                    for i in range(4):
                        fns[4 * c + i] = (t_, i)
                return fns, list(range(NT))

            def load_q2(cout, half, pool, tag, eng):
                """Combined cout [4096,32]; 2 chunks (s-halves): chunk s
                holds kt = 2r+s for r in 0..7."""
                fns = {}
                order = []
                src = cout.rearrange("(r h s p) b -> h s p r b",
                                     h=2, s=2, p=128)[half]
                for s in range(2):
                    t_ = hq8p.tile([128, 8 * NB], BF16, tag=tag + "8")
                    eng.dma_start(
                        t_[:].rearrange("p (r b) -> p r b", r=8),
                        src[s])
                    for r_ in range(8):
                        kt = 2 * r_ + s
                        fns[kt] = (t_, r_)
                        order.append(kt)
                return fns, order

            def fns_lhs(fns):
                def f(kt):
                    t_, i = fns[kt]
                    return t_[:, i * NB:i * NB + NB]
                return f

            def assemble_full(fns, pool, name):
                # 4 contiguous-kt chunk tiles -> one [128, NT*NB] tile
                ft = pool.tile([128, NT * NB], BF16, tag="convf",
                               name=name)
                for c in range(4):
                    t_ = fns[4 * c][0]
                    nc.vector.tensor_copy(
                        ft[:, c * 4 * NB:(c + 1) * 4 * NB], t_[:])
                return ft

            # ---------------- bulk input matmuls ----------------
            def finish_bulk(psA, psB, bq_ap, nm):
                u_sb = usbp.tile([128, GL], F32, tag="usb", name=nm + "u")
                nc.vector.tensor_copy(u_sb[:, 0:512], psA[:])
                nc.vector.tensor_copy(u_sb[:, 512:GL], psB[:])
                uts = []
                for t in range(4):
                    uq = uqp.tile([128, SL], F32, tag="uq",
                                  name=f"{nm}uq{t}")
                    for q in range(4):
                        nc.sync.dma_start(
                            uq[32 * q:32 * q + 32, :],
                            u_sb[32 * t:32 * t + 32,
                                 SL * q:SL * q + SL])
                    nc.vector.tensor_add(uq[:], uq[:], bq_ap)
                    uts.append(uq)
                return uts

            def bulk(x_sb, rhs_fn, bq_ap, nm):
                psA = psbp.tile([128, 512], F32, tag="psb", name=nm + "A")
                psB = psbp.tile([128, 512], F32, tag="psb", name=nm + "B")
                for kt in range(NT):
                    r0, r1 = rhs_fn(kt)
                    lhs = x_sb[:, kt * 128:(kt + 1) * 128]
                    nc.tensor.matmul(psA[:], lhs, r0,
                                     start=(kt == 0), stop=(kt == NT - 1),
                                     skip_group_check=True)
                    nc.tensor.matmul(psB[:], lhs, r1,
                                     start=(kt == 0), stop=(kt == NT - 1),
                                     skip_group_check=True)
                return finish_bulk(psA, psB, bq_ap, nm)

            # ---------------- conv emit (from gathered h2 pairs) --------
            def emit_conv(cwt_sb, cb_sb, conv_tiles):
                b1 = [conv_tiles[0], conv_tiles[1], conv_tiles[2],
                      conv_tiles[3]]
                b2 = [conv_tiles[3], conv_tiles[2], conv_tiles[1],
                      conv_tiles[0]]
                for w in range(4):
                    pcv = pcvp.tile([128, 512], F32, tag="pcv",
                                    name=f"pcv{w}")
                    first = True
                    for br, src in ((0, b1[w]), (1, b2[w])):
                        lhs = src[:].rearrange("p (kt b) -> p kt b", kt=NT)
                        for j in range(4):
                            st = cvlp.tile([128, 64], BF16, tag="cvl",
                                           name=f"cvl{w}_{br}_{j}")
                            nc.vector.tensor_copy(
                                st[:].rearrange("p (h b) -> p h b", h=4),
                                lhs[:, j::4, 16 * br:16 * br + 16])
                            nc.tensor.matmul(
                                pcv[0:64, :], st[:],
                                cwt_sb[:, (4 * br + j) * C:
                                       (4 * br + j + 1) * C],
                                start=first, stop=(br == 1 and j == 3),
                                skip_group_check=True)
                            first = False
                    cvs = usbp.tile([64, C], F32, tag="cvs", name=f"cvs{w}")
                    nc.vector.tensor_add(cvs[:], pcv[0:64, :], cb_sb[:])
                    cvo = usbp.tile([64, C], F32, tag="cvo", name=f"cvo{w}")
                    nc.vector.tensor_scalar_mul(cvo[:], cvs[:], 0.2)
                    nc.vector.tensor_max(cvo[:], cvo[:], cvs[:])
                    nc.sync.dma_start(convout[w], cvo[:])

            # =========================================================
            # Preamble: input/weight DMAs on the two HWDGE queues
            # =========================================================
            xe_sb = xinp.tile([128, NT * 128], BF16, tag="xin", name="xe")
            nc.scalar.dma_start(xe_sb[:], xET[:, :])

            def load_bq(src, l, name):
                t_ = bqp.tile([128, SL], F32, tag="bq", name=name)
                nc.scalar.dma_start(t_[:], src[l])
                return t_

            eb0 = load_bq(eBq, 0, "eb0")
            db0 = load_bq(dBq, 0, "db0")

            ones_sb = brp.tile([1, NB], BF16, tag="ones", name="ones_sb")
            nc.scalar.dma_start(ones_sb[:], ones[:, :])

            def load_br(src, l, name):
                t_ = brp.tile([1, GL], BF16, tag="br", name=name)
                nc.scalar.dma_start(t_[:], src[l])
                return t_

            ebr1 = load_br(eBr, 1, "ebr1")
            dbr0 = load_br(dBr, 0, "dbr0")
            dbr1 = load_br(dBr, 1, "dbr1")

            # all preamble DMAs on the scalar queue: the critical E0 path
            # (x + eWih0 stream + bulk) gets it first at full bandwidth;
            # sync stays free for per-step DMAs.
            wstr_tiles = []
            for kt in range(NT):
                wt = wstrp.tile([128, GL], BF16, tag="wstr", name=f"ws{kt}")
                nc.scalar.dma_start(
                    wt[:], eWih[0, kt * 128:(kt + 1) * 128, :])
                wstr_tiles.append(wt)
            ue = bulk(xe_sb,
                      lambda kt: (wstr_tiles[kt][:, 0:512],
                                  wstr_tiles[kt][:, 512:GL]),
                      eb0[:], "ue")

            whh_e0 = load_w(eWhh, 0, "whh_e0", nc.scalar)
            wih_e1 = load_w(eWih, 1, "wih_e1", nc.scalar)
            whh_e1 = load_w(eWhh, 1, "whh_e1", nc.scalar)
            wih_d0 = load_w(dWih, 0, "wih_d0", nc.scalar)
            xd_sb = xinp.tile([128, NT * 128], BF16, tag="xin", name="xd")
            nc.scalar.dma_start(xd_sb[:], xDT[:, :])
            whh_d0 = load_w(dWhh, 0, "whh_d0", nc.scalar)
            wih_d1 = load_w(dWih, 1, "wih_d1", nc.scalar)
            whh_d1 = load_w(dWhh, 1, "whh_d1", nc.scalar)

            cb_sb = bqp.tile([64, C], F32, tag="cb", name="cb_sb")
            nc.scalar.dma_start(cb_sb[:], cB[:])
            cwt_sb = cwtp.tile([128, 8 * C], BF16, tag="cwt")
            nc.scalar.dma_start(
                cwt_sb[:].rearrange("p (j o) -> p j o", j=8),
                cWT.rearrange("(j p) o -> p j o", p=128))

            # =========================================================
            # Phase E + C: wavefront, one combined AG per time-step
            # =========================================================
            h1big = None      # gathered h1_t chunk pair
            h2big = None      # gathered h2_{t-1} chunk pair
            c1 = c2 = None
            cin_next = None
            ud = None

            for phase in ("E", "C"):
                if phase == "E":
                    uts, w0, w1i, w1h, br1 = ue, whh_e0, wih_e1, whh_e1, ebr1
                else:
                    uts, w0, w1i, w1h, br1 = ud, whh_d0, wih_d1, whh_d1, dbr1

                for t in range(4):
                    first = (phase == "E" and t == 0)
                    # ---- layer 0 step t ----
                    if first:
                        c1, ht1 = cellq(uts[0][:], None, None, "1")
                    else:
                        P0 = Pp.tile([128, SL], F32, tag="P",
                                     name=f"P0{phase}{t}")
                        qmm(P0, fns_lhs(h1big), w0, start=True, stop=True,
                            order=h1ord)
                        c1, ht1 = cellq(P0[:], uts[t][:], c1, "1")
                    if first:
                        cin = dramp.tile([SL, NB], BF16, tag="cin1",
                                         name="cinE0")
                    else:
                        cin = cin_next
                    write_ht(cin, ht1, 0)
                    cout = ag(cin, SL if first else 2 * SL, f"{phase}{t}")

                    # dec bulk (reads resident dWih0) rides the E windows
                    if phase == "E" and t == 2:
                        ud = bulk(xd_sb,
                                  lambda kt: (wih_d0[:, kt * GL:
                                                     kt * GL + 512],
                                              wih_d0[:, kt * GL + 512:
                                                     (kt + 1) * GL]),
                                  db0[:], "ud")

                    # ---- AG results ----
                    if first:
                        h1big, h1ord = load_q1(cout, h1bigp, "h1big")
                    else:
                        h1big, h1ord = load_q2(cout, 0, h1bigp, "h1big",
                                               nc.sync)
                        h2big, h2ord = load_q2(cout, 1, h2bigp, "h2big",
                                               nc.scalar)

                    # ---- layer 1 step t ----
                    P1 = Pp.tile([128, SL], F32, tag="P",
                                 name=f"P1{phase}{t}")
                    bias_mm(P1, br1)
                    if first:
                        qmm(P1, fns_lhs(h1big), w1i, start=False,
                            stop=True, order=h1ord)
                        c2, ht2 = cellq(P1[:], None, None, "2")
                    else:
                        qmm(P1, fns_lhs(h2big), w1h, start=False,
                            stop=False, order=h2ord)
                        qmm(P1, fns_lhs(h1big), w1i, start=False,
                            stop=True, order=h1ord)
                        oi = 0 if (phase == "C" and t == 3) else None
                        c2, ht2 = cellq(P1[:], None, c2, "2", out_idx=oi)
                    # write h2_t into the NEXT AG's cin
                    if phase == "C" and t == 3:
                        cin_next = dramp.tile([SL, NB], BF16, tag="cin1",
                                              name="cinAR0")
                        write_ht(cin_next, ht2, 0)
                    else:
                        cin_next = dramp.tile([2 * SL, NB], BF16,
                                              tag="cin2",
                                              name=f"cin{phase}{t + 1}")
                        write_ht(cin_next, ht2, 1)

            # =========================================================
            # Phase AR: alternating single-h AllGathers
            # =========================================================
            conv_tiles = []
            for t in range(NAR):
                # ---- AG a(t): gather h2_{t-1} ----
                P0 = Pp.tile([128, SL], F32, tag="P", name=f"P0a{t}")
                bias_mm(P0, dbr0)
                qmm(P0, fns_lhs(h1big), whh_d0, start=False, stop=False,
                    order=h1ord)
                cout = ag(cin_next, SL, f"a{t}")
                h2big, h2ord = load_q1(cout, h2bigp, "h2big")
                if t < 4:
                    conv_tiles.append(
                        assemble_full(h2big, convfp, f"cf{t}"))
                qmm(P0, fns_lhs(h2big), wih_d0, start=False, stop=True,
                    order=h2ord)
                c1, ht1 = cellq(P0[:], None, c1, "1")
                cin_next = dramp.tile([SL, NB], BF16, tag="cin1",
                                      name=f"cinb{t}")
                write_ht(cin_next, ht1, 0)

                # ---- AG b(t): gather h1 of this step ----
                P1 = Pp.tile([128, SL], F32, tag="P", name=f"P1a{t}")
                bias_mm(P1, dbr1)
                qmm(P1, fns_lhs(h2big), whh_d1, start=False, stop=False,
                    order=h2ord)
                cout = ag(cin_next, SL, f"b{t}")
                if t == 3:
                    emit_conv(cwt_sb, cb_sb, conv_tiles)
                h1big, h1ord = load_q1(cout, h1bigp, "h1big")
                qmm(P1, fns_lhs(h1big), wih_d1, start=False, stop=True,
                    order=h1ord)
                c2, ht2 = cellq(P1[:], None, c2, "2", out_idx=t + 1)
                if t < NAR - 1:
                    cin_next = dramp.tile([SL, NB], BF16, tag="cin1",
                                          name=f"cina{t + 1}")
                    write_ht(cin_next, ht2, 0)

    nc.compile()
    return nc


def _prep_inputs(x1, x2, enc_Wih, enc_Whh, enc_bih, enc_bhh,
                 dec_Wih, dec_Whh, dec_bih, dec_bhh, conv_W, conv_b):
    def colvecs(x):
        return [np.ascontiguousarray(x[:, :, :, t].reshape(B, L))
                for t in range(4)]

    x1c, x2c = colvecs(x1), colvecs(x2)

    def ximg(xa):
        # [4, L, NB] -> SBUF image [128, kt*128 + t*32 + b]
        return np.ascontiguousarray(
            xa.reshape(4, NT, 128, NB).transpose(2, 1, 0, 3)
            .reshape(128, NT * 4 * NB)).astype(NPBF)

    xET = ximg(np.stack([
        np.concatenate([x2c[t], x1c[3 - t]], axis=0)[:, PERM].T
        for t in range(4)]))
    xDT = ximg(np.stack([
        np.concatenate([x1c[t], x2c[3 - t]], axis=0)[:, PERM].T
        for t in range(4)]))

    def prep_core(k, Wih, Whh, bih, bhh):
        rows = np.concatenate([g * L + PERM[k * SL:(k + 1) * SL]
                               for g in GORDER])
        wihT = np.stack([np.ascontiguousarray(Wih[l][rows][:, PERM].T)
                         for l in range(2)])
        whhT = np.stack([np.ascontiguousarray(Whh[l][rows][:, PERM].T)
                         for l in range(2)])
        bb = np.stack([(bih[l] + bhh[l])[rows] for l in range(2)])
        # quadrant layout: [l, 4 gates x 32 batch, SL]
        bq = np.broadcast_to(
            bb.reshape(2, 4, 1, SL), (2, 4, 32, SL)).reshape(2, 128, SL)
        br = bb.reshape(2, 1, GL)
        return (wihT.astype(NPBF), whhT.astype(NPBF),
                np.ascontiguousarray(bq).astype(np.float32),
                np.ascontiguousarray(br).astype(NPBF))

    cWT = np.ascontiguousarray(conv_W.T).astype(NPBF)
    cBr = np.broadcast_to(conv_b[None, :], (64, C)).copy().astype(np.float32)
    ones_h = np.ones((1, NB), dtype=NPBF)

    in_maps = []
    for k in range(NCORES):
        eWihT, eWhhT, eBq_, eBr_ = prep_core(
            k, enc_Wih, enc_Whh, enc_bih, enc_bhh)
        dWihT, dWhhT, dBq_, dBr_ = prep_core(
            k, dec_Wih, dec_Whh, dec_bih, dec_bhh)
        in_maps.append({
            "xET": xET, "xDT": xDT,
            "eWih": eWihT, "eWhh": eWhhT, "eBq": eBq_, "eBr": eBr_,
            "dWih": dWihT, "dWhh": dWhhT, "dBq": dBq_, "dBr": dBr_,
            "ones": ones_h, "cWT": cWT, "cB": cBr,
        })
    return in_maps


def _postprocess(results, x1, x2):
    chunks = np.zeros((8, B * 2, L), np.float32)
    for k in range(NCORES):
        chunks[:, :, k * SL:(k + 1) * SL] = results[k]["chunks_out"]
    convout = results[0]["convout"]

    def tochunk(t, half):
        v = chunks[t, half * B:(half + 1) * B, :]
        return v[:, IPERM].reshape(B, C, H)

    de1 = tochunk(0, 0)
    p1 = [tochunk(1 + j, 0) for j in range(NAR)]
    de2 = tochunk(0, 1)
    p2 = [tochunk(1 + j, 1) for j in range(NAR)]

    mid1 = np.stack([de1, p1[0], p1[1], p1[2]], axis=-1)
    tail1 = np.stack([p1[3], p1[4], p1[5], p1[6]], axis=-1)
    head2 = np.stack([p2[6], p2[5], p2[4], p2[3]], axis=-1)
    mid2 = np.stack([p2[2], p2[1], p2[0], de2], axis=-1)

    out = convout.reshape(4, 4, B, C).transpose(2, 3, 1, 0)
    out = np.ascontiguousarray(out, dtype=np.float32)
    return (out, np.asarray(x1), mid1, tail1, head2, mid2, np.asarray(x2))


def _run(in_maps, trace=False):
    if "nc" not in _CACHE:
        _CACHE["nc"] = _build_nc()
        _CACHE["tmpdir"] = tempfile.mkdtemp(prefix="lstmk_")
    nc = _CACHE["nc"]
    res = bass_utils.run_bass_kernel_spmd(
        nc, in_maps, core_ids=list(range(NCORES)), trace=trace,
        tmpdir=_CACHE["tmpdir"] if trace else None)
    return res


def kernel(**inputs):
    inputs = {k: np.asarray(v, dtype=np.float32) for k, v in inputs.items()}
    in_maps = _prep_inputs(**inputs)
    res = _run(in_maps, trace=False)
    return _postprocess(res.results, inputs["x1"], inputs["x2"])


def kernel_traced(**inputs):
    inputs = {k: np.asarray(v, dtype=np.float32) for k, v in inputs.items()}
    in_maps = _prep_inputs(**inputs)
    res = _run(in_maps, trace=True)
    return _postprocess(res.results, inputs["x1"], inputs["x2"]), res
